# revision 2
# baseline (speedup 1.0000x reference)
"""MultiLevelAlignedRoIPooling Trainium2 kernel.

Strategy
--------
Output[b, n, i, j, c] = sum_{a,b' in {0,1}} w_ab' * feat_lvl[b, y_a(i), x_b'(j), c]
(7x7 aligned bilinear RoI pooling; the 2x2 "avg pool" in the reference is
algebraically the 4-tap bilinear interpolation at each of the 7x7 sample
points).

With the reference's box distribution (h, w in [32, 400] => area_sqrt in
[32, 400] c [32, 448)), *every* box is assigned pyramid level 4, i.e. all
gathers read feat0 only.  We verify this on the host and fall back to a
plain numpy replica of the reference in the (impossible) general case.

Sharding: 8 cores = 4 batches x 2 halves of the 256 boxes.  Each core:
  - dma_gather (gpsimd SWDGE) fetches, for every sample point, the two
    2-pixel-wide row segments (y_lo row and y_hi row, 512 f32 = 2KB each)
    straight from its batch's feat0 in HBM, landing sample points on
    partitions (partition = box, free slot = sample index chunk).
  - DVE combines the 4 taps with per-partition scalar weights
    (tensor_scalar + scalar_tensor_tensor chain).
  - Results stream back to DRAM as [box, 49*256] rows.

Host prep computes gather indices (int16) + tap weights (f32) with numpy
f32 math that mirrors the reference op-for-op.
"""

import numpy as np

B, N, C = 4, 256, 256
H = W = 128
OUT = 7
NS = OUT * OUT            # 49 sample points per box
BOX_PER_CORE = 128
NCORES = 8
CH = 7                    # sample slots per chunk
NCHUNK = NS // CH         # 7
NIDX = BOX_PER_CORE * NS  # 6272 gathers per tap per core
WCOLS = NIDX // 16        # 392 wrapped index columns

_NC_CACHE = None


def _build_nc():
    """Build + compile the per-core Bass program (same program on all cores)."""
    global _NC_CACHE
    if _NC_CACHE is not None:
        return _NC_CACHE
    from contextlib import ExitStack

    import concourse.bass as bass
    import concourse.tile as tile
    from concourse import bacc, mybir

    f32 = mybir.dt.float32
    i16 = mybir.dt.int16
    mult = mybir.AluOpType.mult
    add = mybir.AluOpType.add

    nc = bacc.Bacc("TRN2", target_bir_lowering=False, debug=False, num_devices=NCORES)
    feat = nc.dram_tensor("feat", [H * W, C], f32, kind="ExternalInput")
    idx = nc.dram_tensor("idx", [2, 128, WCOLS], i16, kind="ExternalInput")
    wts = nc.dram_tensor("wts", [128, 4 * NS], f32, kind="ExternalInput")
    out = nc.dram_tensor("out", [128, NS * C], f32, kind="ExternalOutput")

    with tile.TileContext(nc) as tc, ExitStack() as ctx:
        meta = ctx.enter_context(tc.tile_pool(name="meta", bufs=1))
        gp = ctx.enter_context(tc.tile_pool(name="g", bufs=3))
        op = ctx.enter_context(tc.tile_pool(name="o", bufs=3))

        idx_t = meta.tile([128, 2 * WCOLS], i16, name="idx_t")
        nc.sync.dma_start(idx_t[:, 0:WCOLS], idx.ap()[0])
        nc.sync.dma_start(idx_t[:, WCOLS : 2 * WCOLS], idx.ap()[1])
        wts_t = meta.tile([128, 4 * NS], f32, name="wts_t")
        nc.sync.dma_start(wts_t[:], wts.ap()[:, :])

        # Gather source: overlapping 2-pixel windows, one per pixel row index.
        feat_gap = bass.AP(feat, 0, [[C, H * W - 1], [1, 2 * C]])

        ncol = CH * 128 // 16  # wrapped idx columns per chunk
        for c in range(NCHUNK):
            g0 = gp.tile([128, CH, 2 * C], f32, tag="g0", name=f"g0_{c}")
            g1 = gp.tile([128, CH, 2 * C], f32, tag="g1", name=f"g1_{c}")
            nc.gpsimd.dma_gather(
                g0[:],
                feat_gap,
                idx_t[:, c * ncol : (c + 1) * ncol],
                num_idxs=CH * 128,
                num_idxs_reg=CH * 128,
                elem_size=2 * C,
                elem_step=C,
            )
            nc.gpsimd.dma_gather(
                g1[:],
                feat_gap,
                idx_t[:, WCOLS + c * ncol : WCOLS + (c + 1) * ncol],
                num_idxs=CH * 128,
                num_idxs_reg=CH * 128,
                elem_size=2 * C,
                elem_step=C,
            )
            och = op.tile([128, CH * C], f32, tag="och", name=f"och_{c}")
            for kl in range(CH):
                k = c * CH + kl
                tmp = op.tile([128, C], f32, tag="tmp", name=f"tmp_{c}_{kl}")
                nc.vector.tensor_scalar(
                    tmp[:], g0[:, kl, 0:C], wts_t[:, 0 * NS + k : 0 * NS + k + 1],
                    None, mult,
                )
                nc.vector.scalar_tensor_tensor(
                    tmp[:], g0[:, kl, C : 2 * C],
                    wts_t[:, 1 * NS + k : 1 * NS + k + 1], tmp[:], mult, add,
                )
                nc.vector.scalar_tensor_tensor(
                    tmp[:], g1[:, kl, 0:C],
                    wts_t[:, 2 * NS + k : 2 * NS + k + 1], tmp[:], mult, add,
                )
                nc.vector.scalar_tensor_tensor(
                    och[:, kl * C : (kl + 1) * C], g1[:, kl, C : 2 * C],
                    wts_t[:, 3 * NS + k : 3 * NS + k + 1], tmp[:], mult, add,
                )
            nc.sync.dma_start(out.ap()[:, c * CH * C : (c + 1) * CH * C], och[:])

    nc.compile()
    _NC_CACHE = nc
    return nc


def _host_tables(boxes):
    """Numpy f32 replica of the reference's index/weight math.

    Returns None if any box is assigned a level other than 4 (never happens
    with the reference's input distribution), else per-core gather tables.
    """
    f32 = np.float32
    b = boxes.astype(f32)
    box_h = b[..., 2] - b[..., 0]
    box_w = b[..., 3] - b[..., 1]
    area = np.sqrt(box_h * box_w)
    with np.errstate(divide="ignore", invalid="ignore"):
        lev = np.floor(np.log(area / f32(224.0)) / np.log(f32(2.0))) + f32(4.0)
    if not np.all(np.isfinite(lev)):
        return None
    levels = np.clip(lev.astype(np.int32), 4, 64)
    if not np.all(levels == 4):
        return None
    scale = np.exp2(levels.astype(f32))
    bs = b / scale[..., None]
    bh = (box_h / scale).astype(f32)
    bw = (box_w / scale).astype(f32)
    by = (bs[..., 0] - f32(0.5)).astype(f32)
    bx = (bs[..., 1] - f32(0.5)).astype(f32)
    offs = ((np.arange(OUT, dtype=f32) + f32(0.5)) / f32(OUT)).astype(f32)
    gy = (by[..., None] + offs * bh[..., None]).astype(f32)  # [B,N,7]
    gx = (bx[..., None] + offs * bw[..., None]).astype(f32)
    y0 = np.maximum(f32(0.0), np.floor(gy))
    x0 = np.maximum(f32(0.0), np.floor(gx))
    bnd = f32(H - 1)
    y_lo = np.minimum(y0, bnd).astype(np.int32)
    y_hi = np.minimum(y0 + f32(1.0), bnd).astype(np.int32)
    x_lo = np.minimum(x0, bnd).astype(np.int32)
    x_hi = np.minimum(x0 + f32(1.0), bnd).astype(np.int32)
    ly = (gy - y0).astype(f32)
    lx = (gx - x0).astype(f32)
    hy = (f32(1.0) - ly).astype(f32)
    hx = (f32(1.0) - lx).astype(f32)
    # 2-pixel gather base in x; remap x-tap weights onto (xb, xb+1)
    xb = np.minimum(x_lo, W - 2)
    wx0 = hx * (x_lo == xb) + lx * (x_hi == xb)
    wx1 = hx * (x_lo == xb + 1) + lx * (x_hi == xb + 1)
    return y_lo, y_hi, xb, hy, ly, wx0.astype(f32), wx1.astype(f32)


def _percore_inputs(feat0, tables, core):
    y_lo, y_hi, xb, hy, ly, wx0, wx1 = tables
    bat, half = divmod(core, 2)
    sl = slice(half * BOX_PER_CORE, (half + 1) * BOX_PER_CORE)
    ylo = y_lo[bat, sl]  # [128, 7]
    yhi = y_hi[bat, sl]
    xbs = xb[bat, sl]
    # flat pixel index of the 2-pixel segment, [128 box, 7 i, 7 j]
    i0 = (ylo[:, :, None] * W + xbs[:, None, :]).astype(np.int32)
    i1 = (yhi[:, :, None] * W + xbs[:, None, :]).astype(np.int32)

    def wrap(ixx):
        # gather sequence i = k*128 + box  (k = i_s*7 + j_s)
        seq = np.transpose(ixx, (1, 2, 0)).reshape(NIDX).astype(np.int16)
        wr = seq.reshape(WCOLS, 16).T  # [16, WCOLS]
        return np.tile(wr, (8, 1))     # replicate across the 8 gpsimd cores

    idx = np.stack([wrap(i0), wrap(i1)])  # [2, 128, 392] int16

    hys = hy[bat, sl]
    lys = ly[bat, sl]
    wx0s = wx0[bat, sl]
    wx1s = wx1[bat, sl]
    w00 = (hys[:, :, None] * wx0s[:, None, :]).reshape(128, NS)
    w01 = (hys[:, :, None] * wx1s[:, None, :]).reshape(128, NS)
    w10 = (lys[:, :, None] * wx0s[:, None, :]).reshape(128, NS)
    w11 = (lys[:, :, None] * wx1s[:, None, :]).reshape(128, NS)
    wts = np.concatenate([w00, w01, w10, w11], axis=1).astype(np.float32)

    return {
        "feat": np.ascontiguousarray(feat0[bat].reshape(H * W, C)),
        "idx": np.ascontiguousarray(idx),
        "wts": np.ascontiguousarray(wts),
    }


def _reference_numpy(feats, boxes):
    """Generic fallback: straight numpy port of the reference (never used
    with the reference input distribution; kept for safety)."""
    f32 = np.float32
    L = len(feats)
    padded = np.zeros((B, L, H, W, C), dtype=f32)
    for i, f in enumerate(feats):
        padded[:, i, : f.shape[1], : f.shape[2], :] = f
    b = boxes.astype(f32)
    box_h = b[..., 2] - b[..., 0]
    box_w = b[..., 3] - b[..., 1]
    area = np.sqrt(box_h * box_w)
    lev = np.floor(np.log(area / f32(224.0)) / np.log(f32(2.0))) + f32(4.0)
    levels = np.clip(lev.astype(np.int32), 4, 64)
    scale = np.exp2(levels.astype(f32))
    bs = b / scale[..., None]
    bh = box_h / scale
    bw = box_w / scale
    yxhw = np.concatenate([bs[..., 0:2], bh[..., None], bw[..., None]], axis=-1)
    lvl = levels - 4
    strides = np.exp2(lvl.astype(f32))
    bnd_h = H / strides - f32(1.0)
    bnd_w = W / strides - f32(1.0)
    by = bnd_w[..., None]  # faithful swap from the reference
    bx = bnd_h[..., None]
    box_y = yxhw[..., 0] - f32(0.5)
    box_x = yxhw[..., 1] - f32(0.5)
    offs = (np.arange(OUT, dtype=f32) + f32(0.5)) / f32(OUT)
    gy = box_y[..., None] + offs * yxhw[..., 2:3]
    gx = box_x[..., None] + offs * yxhw[..., 3:4]
    y0 = np.maximum(f32(0.0), np.floor(gy))
    x0 = np.maximum(f32(0.0), np.floor(gx))
    y01 = np.stack([np.minimum(y0, by), np.minimum(y0 + 1, by)], axis=3).reshape(
        B, N, 2 * OUT
    )
    x01 = np.stack([np.minimum(x0, bx), np.minimum(x0 + 1, bx)], axis=3).reshape(
        B, N, 2 * OUT
    )
    yi = y01.astype(np.int32)
    xi = x01.astype(np.int32)
    bi = np.arange(B)[:, None, None, None]
    li = np.clip(lvl, 0, L - 1)[:, :, None, None]
    gathered = padded[bi, li, yi[:, :, :, None], xi[:, :, None, :]]
    ly = gy - y0
    lx = gx - x0
    hy = 1.0 - ly
    hx = 1.0 - lx
    ky = np.stack([hy, ly], axis=3).reshape(B, N, 2 * OUT, 1)
    kx = np.stack([hx, lx], axis=3).reshape(B, N, 1, 2 * OUT)
    kern = (ky * kx * 4.0).astype(f32)
    weighted = gathered * kern[..., None]
    out = weighted.reshape(B, N, OUT, 2, OUT, 2, C).mean(axis=(3, 5))
    return out.astype(f32)


_TRACE_TMPDIR = None


def _run(in_maps, trace=False):
    from concourse.bass_utils import run_bass_kernel_spmd

    nc = _build_nc()
    kw = {}
    if trace and _TRACE_TMPDIR:
        kw["tmpdir"] = _TRACE_TMPDIR
    return run_bass_kernel_spmd(nc, in_maps, list(range(NCORES)), trace=trace, **kw)


def _kernel_impl(inputs, trace=False):
    feats = [np.asarray(inputs[f"feat{i}"], dtype=np.float32) for i in range(5)]
    boxes = np.asarray(inputs["boxes"], dtype=np.float32)
    tables = _host_tables(boxes)
    if tables is None:
        return _reference_numpy(feats, boxes), None
    in_maps = [_percore_inputs(feats[0], tables, c) for c in range(NCORES)]
    res = _run(in_maps, trace=trace)
    full = np.empty((B, N, OUT, OUT, C), dtype=np.float32)
    for core in range(NCORES):
        bat, half = divmod(core, 2)
        o = res.results[core]["out"].reshape(BOX_PER_CORE, OUT, OUT, C)
        full[bat, half * BOX_PER_CORE : (half + 1) * BOX_PER_CORE] = o
    return full, res


def kernel(**inputs):
    out, _ = _kernel_impl(inputs)
    return out


def kernel_profiled(**inputs):
    """Like kernel() but with trace=True; returns (output, BassKernelResults)."""
    return _kernel_impl(inputs, trace=True)


# revision 6
# speedup vs baseline: 1.4250x; 1.4250x over previous
"""MultiLevelAlignedRoIPooling Trainium2 kernel.

Strategy
--------
Output[b, n, i, j, c] = sum_{a,b' in {0,1}} w_ab' * feat_lvl[b, y_a(i), x_b'(j), c]
(7x7 aligned bilinear RoI pooling; the 2x2 "avg pool" in the reference is
algebraically the 4-tap bilinear interpolation at each of the 7x7 sample
points).

With the reference's box distribution (h, w in [32, 400] => area_sqrt in
[32, 400] c [32, 448)), *every* box is assigned pyramid level 4, i.e. all
gathers read feat0 only.  We verify this on the host and fall back to a
plain numpy replica of the reference in the (impossible) general case.

Sharding: 8 cores = 4 batches x 2 halves of the 256 boxes.  Each core:
  - dma_gather (gpsimd SWDGE) fetches, for every sample point, the two
    2-pixel-wide row segments (y_lo row and y_hi row, 512 f32 = 2KB each)
    straight from its batch's feat0 in HBM, landing sample points on
    partitions (partition = box, free slot = sample index chunk).
  - DVE combines the 4 taps with per-partition scalar weights
    (tensor_scalar + scalar_tensor_tensor chain).
  - Results stream back to DRAM as [box, 49*256] rows.

Host prep computes gather indices (int16) + tap weights (f32) with numpy
f32 math that mirrors the reference op-for-op.
"""

import numpy as np

B, N, C = 4, 256, 256
H = W = 128
OUT = 7
NS = OUT * OUT            # 49 sample points per box
BOX_PER_CORE = 128
NCORES = 8
CH = 7                    # sample slots per chunk
NCHUNK = NS // CH         # 7
NIDX = BOX_PER_CORE * NS  # 6272 gathers per tap per core
WCOLS = NIDX // 16        # 392 wrapped index columns

_NC_CACHE = None


def _build_nc():
    """Build + compile the per-core Bass program (same program on all cores)."""
    global _NC_CACHE
    if _NC_CACHE is not None:
        return _NC_CACHE
    from contextlib import ExitStack

    import concourse.bass as bass
    import concourse.tile as tile
    from concourse import bacc, mybir

    f32 = mybir.dt.float32
    i16 = mybir.dt.int16
    mult = mybir.AluOpType.mult
    add = mybir.AluOpType.add

    nc = bacc.Bacc("TRN2", target_bir_lowering=False, debug=False, num_devices=NCORES)
    # feat_pairs: row r = pixel (y, x) holding [feat[y,x,:], feat[y+1,x,:]]
    feat = nc.dram_tensor("feat", [H * W, 2 * C], f32, kind="ExternalInput")
    idx = nc.dram_tensor("idx", [128, WCOLS], i16, kind="ExternalInput")
    wts = nc.dram_tensor("wts", [128, 4 * NS], f32, kind="ExternalInput")
    out = nc.dram_tensor("out", [128, NS * C], f32, kind="ExternalOutput")

    with tile.TileContext(nc) as tc, ExitStack() as ctx:
        meta = ctx.enter_context(tc.tile_pool(name="meta", bufs=1))
        gp = ctx.enter_context(tc.tile_pool(name="g", bufs=3))
        tp = ctx.enter_context(tc.tile_pool(name="t", bufs=6))
        op = ctx.enter_context(tc.tile_pool(name="o", bufs=3))

        idx_t = meta.tile([128, WCOLS], i16, name="idx_t")
        nc.sync.dma_start(idx_t[:], idx.ap()[:, :])
        wts_t = meta.tile([128, 4 * NS], f32, name="wts_t")
        nc.sync.dma_start(wts_t[:], wts.ap()[:, :])

        # Gather source: one elem = 4KB covering pixels (y,xb),(y,xb+1) with
        # both y/y+1 rows each (row-pair layout), elem_step = one pixel pair.
        feat_gap = bass.AP(feat, 0, [[2 * C, H * W - 1], [1, 4 * C]])

        ncol = CH * 128 // 16  # wrapped idx columns per chunk
        # weight table columns: [wx0 | wx1 | hy | ly] each NS wide
        WX0, WX1, HY, LY = 0, NS, 2 * NS, 3 * NS
        for c in range(NCHUNK):
            # G layout: [128, CH, xtap(2), ytap(2), C]
            g = gp.tile([128, CH, 2, 2, C], f32, tag="g", name=f"g_{c}")
            nc.gpsimd.dma_gather(
                g[:].rearrange("p k x y c -> p k (x y c)"),
                feat_gap,
                idx_t[:, c * ncol : (c + 1) * ncol],
                num_idxs=CH * 128,
                num_idxs_reg=CH * 128,
                elem_size=4 * C,
                elem_step=2 * C,
            )
            och = op.tile([128, CH * C], f32, tag="och", name=f"och_{c}")
            for kl in range(CH):
                k = c * CH + kl
                # x-combine over both y rows: T[ytap, c] = wx0*G[x0] + wx1*G[x1]
                t = tp.tile([128, 2, C], f32, tag="tmp", name=f"t_{c}_{kl}")
                nc.scalar.mul(t[:], g[:, kl, 0, :, :], wts_t[:, WX0 + k : WX0 + k + 1])
                nc.vector.scalar_tensor_tensor(
                    t[:], g[:, kl, 1, :, :], wts_t[:, WX1 + k : WX1 + k + 1],
                    t[:], mult, add,
                )
                # y-combine: O = hy*T[0] + ly*T[1]
                u = tp.tile([128, C], f32, tag="tmpu", name=f"u_{c}_{kl}")
                nc.scalar.mul(u[:], t[:, 1, :], wts_t[:, LY + k : LY + k + 1])
                nc.vector.scalar_tensor_tensor(
                    och[:, kl * C : (kl + 1) * C], t[:, 0, :],
                    wts_t[:, HY + k : HY + k + 1], u[:], mult, add,
                )
            nc.sync.dma_start(out.ap()[:, c * CH * C : (c + 1) * CH * C], och[:])

    nc.compile()
    _NC_CACHE = nc
    return nc


def _host_tables(boxes):
    """Numpy f32 replica of the reference's index/weight math.

    Returns None if any box is assigned a level other than 4 (never happens
    with the reference's input distribution), else per-core gather tables.
    """
    f32 = np.float32
    b = boxes.astype(f32)
    box_h = b[..., 2] - b[..., 0]
    box_w = b[..., 3] - b[..., 1]
    area = np.sqrt(box_h * box_w)
    with np.errstate(divide="ignore", invalid="ignore"):
        lev = np.floor(np.log(area / f32(224.0)) / np.log(f32(2.0))) + f32(4.0)
    if not np.all(np.isfinite(lev)):
        return None
    levels = np.clip(lev.astype(np.int32), 4, 64)
    if not np.all(levels == 4):
        return None
    scale = np.exp2(levels.astype(f32))
    bs = b / scale[..., None]
    bh = (box_h / scale).astype(f32)
    bw = (box_w / scale).astype(f32)
    by = (bs[..., 0] - f32(0.5)).astype(f32)
    bx = (bs[..., 1] - f32(0.5)).astype(f32)
    offs = ((np.arange(OUT, dtype=f32) + f32(0.5)) / f32(OUT)).astype(f32)
    gy = (by[..., None] + offs * bh[..., None]).astype(f32)  # [B,N,7]
    gx = (bx[..., None] + offs * bw[..., None]).astype(f32)
    y0 = np.maximum(f32(0.0), np.floor(gy))
    x0 = np.maximum(f32(0.0), np.floor(gx))
    bnd = f32(H - 1)
    y_lo = np.minimum(y0, bnd).astype(np.int32)
    y_hi = np.minimum(y0 + f32(1.0), bnd).astype(np.int32)
    x_lo = np.minimum(x0, bnd).astype(np.int32)
    x_hi = np.minimum(x0 + f32(1.0), bnd).astype(np.int32)
    ly = (gy - y0).astype(f32)
    lx = (gx - x0).astype(f32)
    hy = (f32(1.0) - ly).astype(f32)
    hx = (f32(1.0) - lx).astype(f32)
    # 2-pixel gather base in x; remap x-tap weights onto (xb, xb+1)
    xb = np.minimum(x_lo, W - 2)
    wx0 = hx * (x_lo == xb) + lx * (x_hi == xb)
    wx1 = hx * (x_lo == xb + 1) + lx * (x_hi == xb + 1)
    return y_lo, y_hi, xb, hy, ly, wx0.astype(f32), wx1.astype(f32)


def _feat_pairs(feat0_b):
    """[H*W, 2*C] row-pair layout: row (y*W+x) = [feat[y,x,:], feat[y+1,x,:]]
    (last row duplicates y=127, matching the reference's boundary clamp)."""
    fp = np.empty((H, W, 2, C), dtype=np.float32)
    fp[:, :, 0] = feat0_b
    fp[:-1, :, 1] = feat0_b[1:]
    fp[-1, :, 1] = feat0_b[-1]
    return np.ascontiguousarray(fp.reshape(H * W, 2 * C))


def _percore_inputs(featp_by_batch, tables, core):
    y_lo, y_hi, xb, hy, ly, wx0, wx1 = tables
    bat, half = divmod(core, 2)
    sl = slice(half * BOX_PER_CORE, (half + 1) * BOX_PER_CORE)
    ylo = y_lo[bat, sl]  # [128, 7]
    xbs = xb[bat, sl]
    # flat pixel index of the 2x2 block base, [128 box, 7 i, 7 j]
    i0 = (ylo[:, :, None] * W + xbs[:, None, :]).astype(np.int32)

    # gather sequence i = k*128 + box  (k = i_s*7 + j_s)
    seq = np.transpose(i0, (1, 2, 0)).reshape(NIDX).astype(np.int16)
    wr = seq.reshape(WCOLS, 16).T  # [16, WCOLS]
    idx = np.tile(wr, (8, 1))      # replicate across the 8 gpsimd cores

    hys = hy[bat, sl]    # [128, 7] per sample-row i
    lys = ly[bat, sl]
    wx0s = wx0[bat, sl]  # [128, 7] per sample-col j
    wx1s = wx1[bat, sl]
    ones_i = np.ones((1, OUT, 1), dtype=np.float32)
    # expand to [128, NS] with k = i*7 + j
    wx0_k = (ones_i * wx0s[:, None, :]).reshape(128, NS)
    wx1_k = (ones_i * wx1s[:, None, :]).reshape(128, NS)
    ones_j = np.ones((1, 1, OUT), dtype=np.float32)
    hy_k = (hys[:, :, None] * ones_j).reshape(128, NS)
    ly_k = (lys[:, :, None] * ones_j).reshape(128, NS)
    wts = np.concatenate([wx0_k, wx1_k, hy_k, ly_k], axis=1).astype(np.float32)

    return {
        "feat": featp_by_batch[bat],
        "idx": np.ascontiguousarray(idx),
        "wts": np.ascontiguousarray(wts),
    }


def _reference_numpy(feats, boxes):
    """Generic fallback: straight numpy port of the reference (never used
    with the reference input distribution; kept for safety)."""
    f32 = np.float32
    L = len(feats)
    padded = np.zeros((B, L, H, W, C), dtype=f32)
    for i, f in enumerate(feats):
        padded[:, i, : f.shape[1], : f.shape[2], :] = f
    b = boxes.astype(f32)
    box_h = b[..., 2] - b[..., 0]
    box_w = b[..., 3] - b[..., 1]
    area = np.sqrt(box_h * box_w)
    lev = np.floor(np.log(area / f32(224.0)) / np.log(f32(2.0))) + f32(4.0)
    levels = np.clip(lev.astype(np.int32), 4, 64)
    scale = np.exp2(levels.astype(f32))
    bs = b / scale[..., None]
    bh = box_h / scale
    bw = box_w / scale
    yxhw = np.concatenate([bs[..., 0:2], bh[..., None], bw[..., None]], axis=-1)
    lvl = levels - 4
    strides = np.exp2(lvl.astype(f32))
    bnd_h = H / strides - f32(1.0)
    bnd_w = W / strides - f32(1.0)
    by = bnd_w[..., None]  # faithful swap from the reference
    bx = bnd_h[..., None]
    box_y = yxhw[..., 0] - f32(0.5)
    box_x = yxhw[..., 1] - f32(0.5)
    offs = (np.arange(OUT, dtype=f32) + f32(0.5)) / f32(OUT)
    gy = box_y[..., None] + offs * yxhw[..., 2:3]
    gx = box_x[..., None] + offs * yxhw[..., 3:4]
    y0 = np.maximum(f32(0.0), np.floor(gy))
    x0 = np.maximum(f32(0.0), np.floor(gx))
    y01 = np.stack([np.minimum(y0, by), np.minimum(y0 + 1, by)], axis=3).reshape(
        B, N, 2 * OUT
    )
    x01 = np.stack([np.minimum(x0, bx), np.minimum(x0 + 1, bx)], axis=3).reshape(
        B, N, 2 * OUT
    )
    yi = y01.astype(np.int32)
    xi = x01.astype(np.int32)
    bi = np.arange(B)[:, None, None, None]
    li = np.clip(lvl, 0, L - 1)[:, :, None, None]
    gathered = padded[bi, li, yi[:, :, :, None], xi[:, :, None, :]]
    ly = gy - y0
    lx = gx - x0
    hy = 1.0 - ly
    hx = 1.0 - lx
    ky = np.stack([hy, ly], axis=3).reshape(B, N, 2 * OUT, 1)
    kx = np.stack([hx, lx], axis=3).reshape(B, N, 1, 2 * OUT)
    kern = (ky * kx * 4.0).astype(f32)
    weighted = gathered * kern[..., None]
    out = weighted.reshape(B, N, OUT, 2, OUT, 2, C).mean(axis=(3, 5))
    return out.astype(f32)


_TRACE_TMPDIR = None


def _run(in_maps, trace=False):
    from concourse.bass_utils import run_bass_kernel_spmd

    nc = _build_nc()
    kw = {}
    if trace and _TRACE_TMPDIR:
        kw["tmpdir"] = _TRACE_TMPDIR
    return run_bass_kernel_spmd(nc, in_maps, list(range(NCORES)), trace=trace, **kw)


def _kernel_impl(inputs, trace=False):
    feats = [np.asarray(inputs[f"feat{i}"], dtype=np.float32) for i in range(5)]
    boxes = np.asarray(inputs["boxes"], dtype=np.float32)
    tables = _host_tables(boxes)
    if tables is None:
        return _reference_numpy(feats, boxes), None
    featp = [_feat_pairs(feats[0][b]) for b in range(B)]
    in_maps = [_percore_inputs(featp, tables, c) for c in range(NCORES)]
    res = _run(in_maps, trace=trace)
    full = np.empty((B, N, OUT, OUT, C), dtype=np.float32)
    for core in range(NCORES):
        bat, half = divmod(core, 2)
        o = res.results[core]["out"].reshape(BOX_PER_CORE, OUT, OUT, C)
        full[bat, half * BOX_PER_CORE : (half + 1) * BOX_PER_CORE] = o
    return full, res


def kernel(**inputs):
    out, _ = _kernel_impl(inputs)
    return out


def kernel_profiled(**inputs):
    """Like kernel() but with trace=True; returns (output, BassKernelResults)."""
    return _kernel_impl(inputs, trace=True)


# revision 9
# speedup vs baseline: 1.4970x; 1.0505x over previous
"""MultiLevelAlignedRoIPooling Trainium2 kernel.

Strategy
--------
Output[b, n, i, j, c] = sum_{a,b' in {0,1}} w_ab' * feat_lvl[b, y_a(i), x_b'(j), c]
(7x7 aligned bilinear RoI pooling; the 2x2 "avg pool" in the reference is
algebraically the 4-tap bilinear interpolation at each of the 7x7 sample
points).

With the reference's box distribution (h, w in [32, 400] => area_sqrt in
[32, 400] c [32, 448)), *every* box is assigned pyramid level 4, i.e. all
gathers read feat0 only.  We verify this on the host and fall back to a
plain numpy replica of the reference in the (impossible) general case.

Sharding: 8 cores = 4 batches x 2 halves of the 256 boxes.  Each core:
  - dma_gather (gpsimd SWDGE) fetches, for every sample point, the two
    2-pixel-wide row segments (y_lo row and y_hi row, 512 f32 = 2KB each)
    straight from its batch's feat0 in HBM, landing sample points on
    partitions (partition = box, free slot = sample index chunk).
  - DVE combines the 4 taps with per-partition scalar weights
    (tensor_scalar + scalar_tensor_tensor chain).
  - Results stream back to DRAM as [box, 49*256] rows.

Host prep computes gather indices (int16) + tap weights (f32) with numpy
f32 math that mirrors the reference op-for-op.
"""

import numpy as np

B, N, C = 4, 256, 256
H = W = 128
OUT = 7
NS = OUT * OUT            # 49 sample points per box
BOX_PER_CORE = 128
NCORES = 8
CHUNKS = (2, 8, 8, 8, 8, 8, 7)  # sample slots per gather chunk (sum = NS)
CHMAX = max(CHUNKS)
NIDX = BOX_PER_CORE * NS  # 6272 gathers per tap per core
WCOLS = NIDX // 16        # 392 wrapped index columns

_NC_CACHE = None


def _build_nc():
    """Build + compile the per-core Bass program (same program on all cores)."""
    global _NC_CACHE
    if _NC_CACHE is not None:
        return _NC_CACHE
    from contextlib import ExitStack

    import concourse.bass as bass
    import concourse.tile as tile
    from concourse import bacc, mybir

    f32 = mybir.dt.float32
    i16 = mybir.dt.int16
    mult = mybir.AluOpType.mult
    add = mybir.AluOpType.add

    nc = bacc.Bacc("TRN2", target_bir_lowering=False, debug=False, num_devices=NCORES)
    # feat_pairs: row r = pixel (y, x) holding [feat[y,x,:], feat[y+1,x,:]]
    feat = nc.dram_tensor("feat", [H * W, 2 * C], f32, kind="ExternalInput")
    idx = nc.dram_tensor("idx", [128, WCOLS], i16, kind="ExternalInput")
    wts = nc.dram_tensor("wts", [128, 4 * NS], f32, kind="ExternalInput")
    out = nc.dram_tensor("out", [128, NS * C], f32, kind="ExternalOutput")

    with tile.TileContext(nc) as tc, ExitStack() as ctx:
        meta = ctx.enter_context(tc.tile_pool(name="meta", bufs=1))
        gp = ctx.enter_context(tc.tile_pool(name="g", bufs=3))
        tp = ctx.enter_context(tc.tile_pool(name="t", bufs=6))
        op = ctx.enter_context(tc.tile_pool(name="o", bufs=3))

        idx_t = meta.tile([128, WCOLS], i16, name="idx_t")
        nc.sync.dma_start(idx_t[:], idx.ap()[:, :])
        wts_t = meta.tile([128, 4 * NS], f32, name="wts_t")
        nc.sync.dma_start(wts_t[:], wts.ap()[:, :])

        # Gather source: one elem = 4KB covering pixels (y,xb),(y,xb+1) with
        # both y/y+1 rows each (row-pair layout), elem_step = one pixel pair.
        feat_gap = bass.AP(feat, 0, [[2 * C, H * W - 1], [1, 4 * C]])

        # weight table columns: [wx0 | wx1 | hy | ly] each NS wide
        WX0, WX1, HY, LY = 0, NS, 2 * NS, 3 * NS
        k0 = 0
        for c, ch in enumerate(CHUNKS):
            # G layout: [128, ch, xtap(2), ytap(2), C]; padded to CHMAX slots
            g = gp.tile([128, CHMAX, 2, 2, C], f32, tag="g", name=f"g_{c}")
            nc.gpsimd.dma_gather(
                g[:, :ch].rearrange("p k x y c -> p k (x y c)"),
                feat_gap,
                idx_t[:, k0 * 8 : (k0 + ch) * 8],
                num_idxs=ch * 128,
                num_idxs_reg=ch * 128,
                elem_size=4 * C,
                elem_step=2 * C,
            )
            och = op.tile([128, CHMAX * C], f32, tag="och", name=f"och_{c}")
            for kl in range(ch):
                k = k0 + kl
                # x-combine over both y rows: T[ytap, c] = wx0*G[x0] + wx1*G[x1]
                t = tp.tile([128, 2, C], f32, tag="tmp", name=f"t_{c}_{kl}")
                nc.scalar.mul(t[:], g[:, kl, 0, :, :], wts_t[:, WX0 + k : WX0 + k + 1])
                nc.vector.scalar_tensor_tensor(
                    t[:], g[:, kl, 1, :, :], wts_t[:, WX1 + k : WX1 + k + 1],
                    t[:], mult, add,
                )
                # y-combine: O = hy*T[0] + ly*T[1]
                u = tp.tile([128, C], f32, tag="tmpu", name=f"u_{c}_{kl}")
                nc.scalar.mul(u[:], t[:, 1, :], wts_t[:, LY + k : LY + k + 1])
                nc.vector.scalar_tensor_tensor(
                    och[:, kl * C : (kl + 1) * C], t[:, 0, :],
                    wts_t[:, HY + k : HY + k + 1], u[:], mult, add,
                )
            nc.sync.dma_start(
                out.ap()[:, k0 * C : (k0 + ch) * C], och[:, : ch * C]
            )
            k0 += ch

    nc.compile()
    _NC_CACHE = nc
    return nc


def _host_tables(boxes):
    """Numpy f32 replica of the reference's index/weight math.

    Returns None if any box is assigned a level other than 4 (never happens
    with the reference's input distribution), else per-core gather tables.
    """
    f32 = np.float32
    b = boxes.astype(f32)
    box_h = b[..., 2] - b[..., 0]
    box_w = b[..., 3] - b[..., 1]
    area = np.sqrt(box_h * box_w)
    with np.errstate(divide="ignore", invalid="ignore"):
        lev = np.floor(np.log(area / f32(224.0)) / np.log(f32(2.0))) + f32(4.0)
    if not np.all(np.isfinite(lev)):
        return None
    levels = np.clip(lev.astype(np.int32), 4, 64)
    if not np.all(levels == 4):
        return None
    scale = np.exp2(levels.astype(f32))
    bs = b / scale[..., None]
    bh = (box_h / scale).astype(f32)
    bw = (box_w / scale).astype(f32)
    by = (bs[..., 0] - f32(0.5)).astype(f32)
    bx = (bs[..., 1] - f32(0.5)).astype(f32)
    offs = ((np.arange(OUT, dtype=f32) + f32(0.5)) / f32(OUT)).astype(f32)
    gy = (by[..., None] + offs * bh[..., None]).astype(f32)  # [B,N,7]
    gx = (bx[..., None] + offs * bw[..., None]).astype(f32)
    y0 = np.maximum(f32(0.0), np.floor(gy))
    x0 = np.maximum(f32(0.0), np.floor(gx))
    bnd = f32(H - 1)
    y_lo = np.minimum(y0, bnd).astype(np.int32)
    y_hi = np.minimum(y0 + f32(1.0), bnd).astype(np.int32)
    x_lo = np.minimum(x0, bnd).astype(np.int32)
    x_hi = np.minimum(x0 + f32(1.0), bnd).astype(np.int32)
    ly = (gy - y0).astype(f32)
    lx = (gx - x0).astype(f32)
    hy = (f32(1.0) - ly).astype(f32)
    hx = (f32(1.0) - lx).astype(f32)
    # 2-pixel gather base in x; remap x-tap weights onto (xb, xb+1)
    xb = np.minimum(x_lo, W - 2)
    wx0 = hx * (x_lo == xb) + lx * (x_hi == xb)
    wx1 = hx * (x_lo == xb + 1) + lx * (x_hi == xb + 1)
    return y_lo, y_hi, xb, hy, ly, wx0.astype(f32), wx1.astype(f32)


def _feat_pairs(feat0_b):
    """[H*W, 2*C] row-pair layout: row (y*W+x) = [feat[y,x,:], feat[y+1,x,:]]
    (last row duplicates y=127, matching the reference's boundary clamp)."""
    fp = np.empty((H, W, 2, C), dtype=np.float32)
    fp[:, :, 0] = feat0_b
    fp[:-1, :, 1] = feat0_b[1:]
    fp[-1, :, 1] = feat0_b[-1]
    return np.ascontiguousarray(fp.reshape(H * W, 2 * C))


def _percore_inputs(featp_by_batch, tables, core):
    y_lo, y_hi, xb, hy, ly, wx0, wx1 = tables
    bat, half = divmod(core, 2)
    sl = slice(half * BOX_PER_CORE, (half + 1) * BOX_PER_CORE)
    ylo = y_lo[bat, sl]  # [128, 7]
    xbs = xb[bat, sl]
    # flat pixel index of the 2x2 block base, [128 box, 7 i, 7 j]
    i0 = (ylo[:, :, None] * W + xbs[:, None, :]).astype(np.int32)

    # gather sequence i = k*128 + box  (k = i_s*7 + j_s)
    seq = np.transpose(i0, (1, 2, 0)).reshape(NIDX).astype(np.int16)
    wr = seq.reshape(WCOLS, 16).T  # [16, WCOLS]
    idx = np.tile(wr, (8, 1))      # replicate across the 8 gpsimd cores

    hys = hy[bat, sl]    # [128, 7] per sample-row i
    lys = ly[bat, sl]
    wx0s = wx0[bat, sl]  # [128, 7] per sample-col j
    wx1s = wx1[bat, sl]
    ones_i = np.ones((1, OUT, 1), dtype=np.float32)
    # expand to [128, NS] with k = i*7 + j
    wx0_k = (ones_i * wx0s[:, None, :]).reshape(128, NS)
    wx1_k = (ones_i * wx1s[:, None, :]).reshape(128, NS)
    ones_j = np.ones((1, 1, OUT), dtype=np.float32)
    hy_k = (hys[:, :, None] * ones_j).reshape(128, NS)
    ly_k = (lys[:, :, None] * ones_j).reshape(128, NS)
    wts = np.concatenate([wx0_k, wx1_k, hy_k, ly_k], axis=1).astype(np.float32)

    return {
        "feat": featp_by_batch[bat],
        "idx": np.ascontiguousarray(idx),
        "wts": np.ascontiguousarray(wts),
    }


def _reference_numpy(feats, boxes):
    """Generic fallback: straight numpy port of the reference (never used
    with the reference input distribution; kept for safety)."""
    f32 = np.float32
    L = len(feats)
    padded = np.zeros((B, L, H, W, C), dtype=f32)
    for i, f in enumerate(feats):
        padded[:, i, : f.shape[1], : f.shape[2], :] = f
    b = boxes.astype(f32)
    box_h = b[..., 2] - b[..., 0]
    box_w = b[..., 3] - b[..., 1]
    area = np.sqrt(box_h * box_w)
    lev = np.floor(np.log(area / f32(224.0)) / np.log(f32(2.0))) + f32(4.0)
    levels = np.clip(lev.astype(np.int32), 4, 64)
    scale = np.exp2(levels.astype(f32))
    bs = b / scale[..., None]
    bh = box_h / scale
    bw = box_w / scale
    yxhw = np.concatenate([bs[..., 0:2], bh[..., None], bw[..., None]], axis=-1)
    lvl = levels - 4
    strides = np.exp2(lvl.astype(f32))
    bnd_h = H / strides - f32(1.0)
    bnd_w = W / strides - f32(1.0)
    by = bnd_w[..., None]  # faithful swap from the reference
    bx = bnd_h[..., None]
    box_y = yxhw[..., 0] - f32(0.5)
    box_x = yxhw[..., 1] - f32(0.5)
    offs = (np.arange(OUT, dtype=f32) + f32(0.5)) / f32(OUT)
    gy = box_y[..., None] + offs * yxhw[..., 2:3]
    gx = box_x[..., None] + offs * yxhw[..., 3:4]
    y0 = np.maximum(f32(0.0), np.floor(gy))
    x0 = np.maximum(f32(0.0), np.floor(gx))
    y01 = np.stack([np.minimum(y0, by), np.minimum(y0 + 1, by)], axis=3).reshape(
        B, N, 2 * OUT
    )
    x01 = np.stack([np.minimum(x0, bx), np.minimum(x0 + 1, bx)], axis=3).reshape(
        B, N, 2 * OUT
    )
    yi = y01.astype(np.int32)
    xi = x01.astype(np.int32)
    bi = np.arange(B)[:, None, None, None]
    li = np.clip(lvl, 0, L - 1)[:, :, None, None]
    gathered = padded[bi, li, yi[:, :, :, None], xi[:, :, None, :]]
    ly = gy - y0
    lx = gx - x0
    hy = 1.0 - ly
    hx = 1.0 - lx
    ky = np.stack([hy, ly], axis=3).reshape(B, N, 2 * OUT, 1)
    kx = np.stack([hx, lx], axis=3).reshape(B, N, 1, 2 * OUT)
    kern = (ky * kx * 4.0).astype(f32)
    weighted = gathered * kern[..., None]
    out = weighted.reshape(B, N, OUT, 2, OUT, 2, C).mean(axis=(3, 5))
    return out.astype(f32)


_TRACE_TMPDIR = None


def _run(in_maps, trace=False):
    from concourse.bass_utils import run_bass_kernel_spmd

    nc = _build_nc()
    kw = {}
    if trace and _TRACE_TMPDIR:
        kw["tmpdir"] = _TRACE_TMPDIR
    return run_bass_kernel_spmd(nc, in_maps, list(range(NCORES)), trace=trace, **kw)


def _kernel_impl(inputs, trace=False):
    feats = [np.asarray(inputs[f"feat{i}"], dtype=np.float32) for i in range(5)]
    boxes = np.asarray(inputs["boxes"], dtype=np.float32)
    tables = _host_tables(boxes)
    if tables is None:
        return _reference_numpy(feats, boxes), None
    featp = [_feat_pairs(feats[0][b]) for b in range(B)]
    in_maps = [_percore_inputs(featp, tables, c) for c in range(NCORES)]
    res = _run(in_maps, trace=trace)
    full = np.empty((B, N, OUT, OUT, C), dtype=np.float32)
    for core in range(NCORES):
        bat, half = divmod(core, 2)
        o = res.results[core]["out"].reshape(BOX_PER_CORE, OUT, OUT, C)
        full[bat, half * BOX_PER_CORE : (half + 1) * BOX_PER_CORE] = o
    return full, res


def kernel(**inputs):
    out, _ = _kernel_impl(inputs)
    return out


def kernel_profiled(**inputs):
    """Like kernel() but with trace=True; returns (output, BassKernelResults)."""
    return _kernel_impl(inputs, trace=True)


# revision 14
# speedup vs baseline: 1.7508x; 1.1696x over previous
"""MultiLevelAlignedRoIPooling Trainium2 kernel.

Strategy
--------
Output[b, n, i, j, c] = sum_{a,b' in {0,1}} w_ab' * feat_lvl[b, y_a(i), x_b'(j), c]
(7x7 aligned bilinear RoI pooling; the 2x2 "avg pool" in the reference is
algebraically the 4-tap bilinear interpolation at each of the 7x7 sample
points).

With the reference's box distribution (h, w in [32, 400] => area_sqrt in
[32, 400] c [32, 448)), *every* box is assigned pyramid level 4, i.e. all
gathers read feat0 only.  We verify this on the host and fall back to a
plain numpy replica of the reference in the (impossible) general case.

Sharding: 8 cores = 4 batches x 2 halves of the 256 boxes.  Each core:
  - dma_gather (gpsimd SWDGE) fetches, for every sample point, the two
    2-pixel-wide row segments (y_lo row and y_hi row, 512 f32 = 2KB each)
    straight from its batch's feat0 in HBM, landing sample points on
    partitions (partition = box, free slot = sample index chunk).
  - DVE combines the 4 taps with per-partition scalar weights
    (tensor_scalar + scalar_tensor_tensor chain).
  - Results stream back to DRAM as [box, 49*256] rows.

Host prep computes gather indices (int16) + tap weights (f32) with numpy
f32 math that mirrors the reference op-for-op.
"""

import os

import numpy as np

# Set KERNEL_FP32=1 to run the gather/combine pipeline in float32 instead
# of float16 (slower, slightly more accurate).
FP32 = os.environ.get("KERNEL_FP32", "0") == "1"
FDT = np.float32 if FP32 else np.float16

B, N, C = 4, 256, 256
H = W = 128
OUT = 7
NS = OUT * OUT            # 49 sample points per box
BOX_PER_CORE = 128
NCORES = 8
CHUNKS = (2, 8, 8, 8, 8, 8, 7)  # sample slots per gather chunk (sum = NS)
CHMAX = max(CHUNKS)
NIDX = BOX_PER_CORE * NS  # 6272 gathers per tap per core
WCOLS = NIDX // 16        # 392 wrapped index columns

_NC_CACHE = None


def _build_nc():
    """Build + compile the per-core Bass program (same program on all cores)."""
    global _NC_CACHE
    if _NC_CACHE is not None:
        return _NC_CACHE
    from contextlib import ExitStack

    import concourse.bass as bass
    import concourse.tile as tile
    from concourse import bacc, mybir

    fdt = mybir.dt.float32 if FP32 else mybir.dt.float16
    i16 = mybir.dt.int16
    mult = mybir.AluOpType.mult
    add = mybir.AluOpType.add

    nc = bacc.Bacc("TRN2", target_bir_lowering=False, debug=False, num_devices=NCORES)
    # feat_pairs: row r = pixel (y, x) holding [feat[y,x,:], feat[y+1,x,:]]
    feat = nc.dram_tensor("feat", [H * W, 2 * C], fdt, kind="ExternalInput")
    idx = nc.dram_tensor("idx", [128, WCOLS], i16, kind="ExternalInput")
    wts = nc.dram_tensor("wts", [128, 4 * NS], mybir.dt.float32, kind="ExternalInput")
    out = nc.dram_tensor("out", [128, NS * C], fdt, kind="ExternalOutput")

    with tile.TileContext(nc) as tc, ExitStack() as ctx:
        meta = ctx.enter_context(tc.tile_pool(name="meta", bufs=1))
        gp = ctx.enter_context(tc.tile_pool(name="g", bufs=4))
        tp = ctx.enter_context(tc.tile_pool(name="t", bufs=10))
        op = ctx.enter_context(tc.tile_pool(name="o", bufs=4))

        idx_t = meta.tile([128, WCOLS], i16, name="idx_t")
        nc.sync.dma_start(idx_t[:], idx.ap()[:, :])
        wts_t = meta.tile([128, 4 * NS], mybir.dt.float32, name="wts_t")
        nc.sync.dma_start(wts_t[:], wts.ap()[:, :])

        # Gather source: one elem covers pixels (y,xb),(y,xb+1) with both
        # y/y+1 rows each (row-pair layout), elem_step = one pixel pair.
        feat_gap = bass.AP(feat, 0, [[2 * C, H * W - 1], [1, 4 * C]])

        # Warmup gather: touches only idx_t; absorbs SWDGE lazy-init cost
        # while the real idx/wts loads are still in flight.
        wu = meta.tile([128, 1, 4 * C], fdt, name="wu")
        wu_idx = meta.tile([128, 8], i16, name="wu_idx")
        nc.gpsimd.memset(wu_idx[:], 0)
        nc.gpsimd.dma_gather(
            wu[:], feat_gap, wu_idx[:], num_idxs=128, num_idxs_reg=128,
            elem_size=4 * C, elem_step=2 * C,
        )

        # weight table columns: [wx0 | wx1 | hy | ly] each NS wide
        WX0, WX1, HY, LY = 0, NS, 2 * NS, 3 * NS
        k0 = 0
        for c, ch in enumerate(CHUNKS):
            # G layout: [128, ch, xtap(2), ytap(2), C]; padded to CHMAX slots
            g = gp.tile([128, CHMAX, 2, 2, C], fdt, tag="g", name=f"g_{c}")
            nc.gpsimd.dma_gather(
                g[:, :ch].rearrange("p k x y c -> p k (x y c)"),
                feat_gap,
                idx_t[:, k0 * 8 : (k0 + ch) * 8],
                num_idxs=ch * 128,
                num_idxs_reg=ch * 128,
                elem_size=4 * C,
                elem_step=2 * C,
            )
            och = op.tile([128, CHMAX * C], fdt, tag="och", name=f"och_{c}")
            for kl in range(ch):
                k = k0 + kl
                # x-combine over both y rows: T[ytap, c] = wx0*G[x0] + wx1*G[x1]
                t = tp.tile([128, 2, C], fdt, tag="tmp", name=f"t_{c}_{kl}")
                nc.scalar.mul(t[:], g[:, kl, 0, :, :], wts_t[:, WX0 + k : WX0 + k + 1])
                nc.vector.scalar_tensor_tensor(
                    t[:], g[:, kl, 1, :, :], wts_t[:, WX1 + k : WX1 + k + 1],
                    t[:], mult, add,
                )
                # y-combine: O = hy*T[0] + ly*T[1]
                u = tp.tile([128, C], fdt, tag="tmpu", name=f"u_{c}_{kl}")
                nc.scalar.mul(u[:], t[:, 1, :], wts_t[:, LY + k : LY + k + 1])
                nc.vector.scalar_tensor_tensor(
                    och[:, kl * C : (kl + 1) * C], t[:, 0, :],
                    wts_t[:, HY + k : HY + k + 1], u[:], mult, add,
                )
            nc.sync.dma_start(
                out.ap()[:, k0 * C : (k0 + ch) * C], och[:, : ch * C]
            )
            k0 += ch

    nc.compile()
    _NC_CACHE = nc
    return nc


def _host_tables(boxes):
    """Numpy f32 replica of the reference's index/weight math.

    Returns None if any box is assigned a level other than 4 (never happens
    with the reference's input distribution), else per-core gather tables.
    """
    f32 = np.float32
    b = boxes.astype(f32)
    box_h = b[..., 2] - b[..., 0]
    box_w = b[..., 3] - b[..., 1]
    area = np.sqrt(box_h * box_w)
    with np.errstate(divide="ignore", invalid="ignore"):
        lev = np.floor(np.log(area / f32(224.0)) / np.log(f32(2.0))) + f32(4.0)
    if not np.all(np.isfinite(lev)):
        return None
    levels = np.clip(lev.astype(np.int32), 4, 64)
    if not np.all(levels == 4):
        return None
    scale = np.exp2(levels.astype(f32))
    bs = b / scale[..., None]
    bh = (box_h / scale).astype(f32)
    bw = (box_w / scale).astype(f32)
    by = (bs[..., 0] - f32(0.5)).astype(f32)
    bx = (bs[..., 1] - f32(0.5)).astype(f32)
    offs = ((np.arange(OUT, dtype=f32) + f32(0.5)) / f32(OUT)).astype(f32)
    gy = (by[..., None] + offs * bh[..., None]).astype(f32)  # [B,N,7]
    gx = (bx[..., None] + offs * bw[..., None]).astype(f32)
    y0 = np.maximum(f32(0.0), np.floor(gy))
    x0 = np.maximum(f32(0.0), np.floor(gx))
    bnd = f32(H - 1)
    y_lo = np.minimum(y0, bnd).astype(np.int32)
    y_hi = np.minimum(y0 + f32(1.0), bnd).astype(np.int32)
    x_lo = np.minimum(x0, bnd).astype(np.int32)
    x_hi = np.minimum(x0 + f32(1.0), bnd).astype(np.int32)
    ly = (gy - y0).astype(f32)
    lx = (gx - x0).astype(f32)
    hy = (f32(1.0) - ly).astype(f32)
    hx = (f32(1.0) - lx).astype(f32)
    # 2-pixel gather base in x; remap x-tap weights onto (xb, xb+1)
    xb = np.minimum(x_lo, W - 2)
    wx0 = hx * (x_lo == xb) + lx * (x_hi == xb)
    wx1 = hx * (x_lo == xb + 1) + lx * (x_hi == xb + 1)
    return y_lo, y_hi, xb, hy, ly, wx0.astype(f32), wx1.astype(f32)


def _feat_pairs(feat0_b):
    """[H*W, 2*C] row-pair layout: row (y*W+x) = [feat[y,x,:], feat[y+1,x,:]]
    (last row duplicates y=127, matching the reference's boundary clamp)."""
    fp = np.empty((H, W, 2, C), dtype=FDT)
    fp[:, :, 0] = feat0_b
    fp[:-1, :, 1] = feat0_b[1:]
    fp[-1, :, 1] = feat0_b[-1]
    return np.ascontiguousarray(fp.reshape(H * W, 2 * C))


def _percore_inputs(featp_by_batch, tables, core):
    y_lo, y_hi, xb, hy, ly, wx0, wx1 = tables
    bat, half = divmod(core, 2)
    sl = slice(half * BOX_PER_CORE, (half + 1) * BOX_PER_CORE)
    ylo = y_lo[bat, sl]  # [128, 7]
    xbs = xb[bat, sl]
    # flat pixel index of the 2x2 block base, [128 box, 7 i, 7 j]
    i0 = (ylo[:, :, None] * W + xbs[:, None, :]).astype(np.int32)

    # gather sequence i = k*128 + box  (k = i_s*7 + j_s)
    seq = np.transpose(i0, (1, 2, 0)).reshape(NIDX).astype(np.int16)
    wr = seq.reshape(WCOLS, 16).T  # [16, WCOLS]
    idx = np.tile(wr, (8, 1))      # replicate across the 8 gpsimd cores

    hys = hy[bat, sl]    # [128, 7] per sample-row i
    lys = ly[bat, sl]
    wx0s = wx0[bat, sl]  # [128, 7] per sample-col j
    wx1s = wx1[bat, sl]
    ones_i = np.ones((1, OUT, 1), dtype=np.float32)
    # expand to [128, NS] with k = i*7 + j
    wx0_k = (ones_i * wx0s[:, None, :]).reshape(128, NS)
    wx1_k = (ones_i * wx1s[:, None, :]).reshape(128, NS)
    ones_j = np.ones((1, 1, OUT), dtype=np.float32)
    hy_k = (hys[:, :, None] * ones_j).reshape(128, NS)
    ly_k = (lys[:, :, None] * ones_j).reshape(128, NS)
    wts = np.concatenate([wx0_k, wx1_k, hy_k, ly_k], axis=1).astype(np.float32)

    return {
        "feat": featp_by_batch[bat],
        "idx": np.ascontiguousarray(idx),
        "wts": np.ascontiguousarray(wts),
    }


def _reference_numpy(feats, boxes):
    """Generic fallback: straight numpy port of the reference (never used
    with the reference input distribution; kept for safety)."""
    f32 = np.float32
    L = len(feats)
    padded = np.zeros((B, L, H, W, C), dtype=f32)
    for i, f in enumerate(feats):
        padded[:, i, : f.shape[1], : f.shape[2], :] = f
    b = boxes.astype(f32)
    box_h = b[..., 2] - b[..., 0]
    box_w = b[..., 3] - b[..., 1]
    area = np.sqrt(box_h * box_w)
    lev = np.floor(np.log(area / f32(224.0)) / np.log(f32(2.0))) + f32(4.0)
    levels = np.clip(lev.astype(np.int32), 4, 64)
    scale = np.exp2(levels.astype(f32))
    bs = b / scale[..., None]
    bh = box_h / scale
    bw = box_w / scale
    yxhw = np.concatenate([bs[..., 0:2], bh[..., None], bw[..., None]], axis=-1)
    lvl = levels - 4
    strides = np.exp2(lvl.astype(f32))
    bnd_h = H / strides - f32(1.0)
    bnd_w = W / strides - f32(1.0)
    by = bnd_w[..., None]  # faithful swap from the reference
    bx = bnd_h[..., None]
    box_y = yxhw[..., 0] - f32(0.5)
    box_x = yxhw[..., 1] - f32(0.5)
    offs = (np.arange(OUT, dtype=f32) + f32(0.5)) / f32(OUT)
    gy = box_y[..., None] + offs * yxhw[..., 2:3]
    gx = box_x[..., None] + offs * yxhw[..., 3:4]
    y0 = np.maximum(f32(0.0), np.floor(gy))
    x0 = np.maximum(f32(0.0), np.floor(gx))
    y01 = np.stack([np.minimum(y0, by), np.minimum(y0 + 1, by)], axis=3).reshape(
        B, N, 2 * OUT
    )
    x01 = np.stack([np.minimum(x0, bx), np.minimum(x0 + 1, bx)], axis=3).reshape(
        B, N, 2 * OUT
    )
    yi = y01.astype(np.int32)
    xi = x01.astype(np.int32)
    bi = np.arange(B)[:, None, None, None]
    li = np.clip(lvl, 0, L - 1)[:, :, None, None]
    gathered = padded[bi, li, yi[:, :, :, None], xi[:, :, None, :]]
    ly = gy - y0
    lx = gx - x0
    hy = 1.0 - ly
    hx = 1.0 - lx
    ky = np.stack([hy, ly], axis=3).reshape(B, N, 2 * OUT, 1)
    kx = np.stack([hx, lx], axis=3).reshape(B, N, 1, 2 * OUT)
    kern = (ky * kx * 4.0).astype(f32)
    weighted = gathered * kern[..., None]
    out = weighted.reshape(B, N, OUT, 2, OUT, 2, C).mean(axis=(3, 5))
    return out.astype(f32)


_TRACE_TMPDIR = None


def _run(in_maps, trace=False):
    from concourse.bass_utils import run_bass_kernel_spmd

    nc = _build_nc()
    kw = {}
    if trace and _TRACE_TMPDIR:
        kw["tmpdir"] = _TRACE_TMPDIR
    return run_bass_kernel_spmd(nc, in_maps, list(range(NCORES)), trace=trace, **kw)


def _kernel_impl(inputs, trace=False):
    feats = [np.asarray(inputs[f"feat{i}"], dtype=np.float32) for i in range(5)]
    boxes = np.asarray(inputs["boxes"], dtype=np.float32)
    tables = _host_tables(boxes)
    if tables is None:
        return _reference_numpy(feats, boxes), None
    featp = [_feat_pairs(feats[0][b]) for b in range(B)]
    in_maps = [_percore_inputs(featp, tables, c) for c in range(NCORES)]
    res = _run(in_maps, trace=trace)
    full = np.empty((B, N, OUT, OUT, C), dtype=np.float32)
    for core in range(NCORES):
        bat, half = divmod(core, 2)
        o = res.results[core]["out"].astype(np.float32).reshape(
            BOX_PER_CORE, OUT, OUT, C
        )
        full[bat, half * BOX_PER_CORE : (half + 1) * BOX_PER_CORE] = o
    return full, res


def kernel(**inputs):
    out, _ = _kernel_impl(inputs)
    return out


def kernel_profiled(**inputs):
    """Like kernel() but with trace=True; returns (output, BassKernelResults)."""
    return _kernel_impl(inputs, trace=True)


# revision 15
# speedup vs baseline: 1.7738x; 1.0131x over previous
"""MultiLevelAlignedRoIPooling Trainium2 kernel.

Strategy
--------
Output[b, n, i, j, c] = sum_{a,b' in {0,1}} w_ab' * feat_lvl[b, y_a(i), x_b'(j), c]
(7x7 aligned bilinear RoI pooling; the 2x2 "avg pool" in the reference is
algebraically the 4-tap bilinear interpolation at each of the 7x7 sample
points).

With the reference's box distribution (h, w in [32, 400] => area_sqrt in
[32, 400] c [32, 448)), *every* box is assigned pyramid level 4, i.e. all
gathers read feat0 only.  We verify this on the host and fall back to a
plain numpy replica of the reference in the (impossible) general case.

Sharding: 8 cores = 4 batches x 2 halves of the 256 boxes.  Each core:
  - dma_gather (gpsimd SWDGE) fetches, for every sample point, the two
    2-pixel-wide row segments (y_lo row and y_hi row, 512 f32 = 2KB each)
    straight from its batch's feat0 in HBM, landing sample points on
    partitions (partition = box, free slot = sample index chunk).
  - DVE combines the 4 taps with per-partition scalar weights
    (tensor_scalar + scalar_tensor_tensor chain).
  - Results stream back to DRAM as [box, 49*256] rows.

Host prep computes gather indices (int16) + tap weights (f32) with numpy
f32 math that mirrors the reference op-for-op.
"""

import os

import numpy as np

# Set KERNEL_FP32=1 to run the gather/combine pipeline in float32 instead
# of float16 (slower, slightly more accurate).
FP32 = os.environ.get("KERNEL_FP32", "0") == "1"
FDT = np.float32 if FP32 else np.float16

B, N, C = 4, 256, 256
H = W = 128
OUT = 7
NS = OUT * OUT            # 49 sample points per box
BOX_PER_CORE = 128
NCORES = 8
CHUNKS = (2, 8, 8, 8, 8, 8, 5, 2)  # sample slots per gather chunk (sum = NS)
CHMAX = max(CHUNKS)
NIDX = BOX_PER_CORE * NS  # 6272 gathers per tap per core
WCOLS = NIDX // 16        # 392 wrapped index columns

_NC_CACHE = None


def _build_nc():
    """Build + compile the per-core Bass program (same program on all cores)."""
    global _NC_CACHE
    if _NC_CACHE is not None:
        return _NC_CACHE
    from contextlib import ExitStack

    import concourse.bass as bass
    import concourse.tile as tile
    from concourse import bacc, mybir

    fdt = mybir.dt.float32 if FP32 else mybir.dt.float16
    i16 = mybir.dt.int16
    mult = mybir.AluOpType.mult
    add = mybir.AluOpType.add

    nc = bacc.Bacc("TRN2", target_bir_lowering=False, debug=False, num_devices=NCORES)
    # feat_pairs: row r = pixel (y, x) holding [feat[y,x,:], feat[y+1,x,:]]
    feat = nc.dram_tensor("feat", [H * W, 2 * C], fdt, kind="ExternalInput")
    idx = nc.dram_tensor("idx", [128, WCOLS], i16, kind="ExternalInput")
    wts = nc.dram_tensor("wts", [128, 4 * NS], mybir.dt.float32, kind="ExternalInput")
    out = nc.dram_tensor("out", [128, NS * C], fdt, kind="ExternalOutput")

    with tile.TileContext(nc) as tc, ExitStack() as ctx:
        meta = ctx.enter_context(tc.tile_pool(name="meta", bufs=1))
        gp = ctx.enter_context(tc.tile_pool(name="g", bufs=4))
        tp = ctx.enter_context(tc.tile_pool(name="t", bufs=10))
        op = ctx.enter_context(tc.tile_pool(name="o", bufs=4))

        idx_t = meta.tile([128, WCOLS], i16, name="idx_t")
        nc.sync.dma_start(idx_t[:], idx.ap()[:, :])
        wts_t = meta.tile([128, 4 * NS], mybir.dt.float32, name="wts_t")
        nc.sync.dma_start(wts_t[:], wts.ap()[:, :])

        # Gather source: one elem covers pixels (y,xb),(y,xb+1) with both
        # y/y+1 rows each (row-pair layout), elem_step = one pixel pair.
        feat_gap = bass.AP(feat, 0, [[2 * C, H * W - 1], [1, 4 * C]])

        # Warmup gather: touches only idx_t; absorbs SWDGE lazy-init cost
        # while the real idx/wts loads are still in flight.
        wu = meta.tile([128, 1, 4 * C], fdt, name="wu")
        wu_idx = meta.tile([128, 8], i16, name="wu_idx")
        nc.gpsimd.memset(wu_idx[:], 0)
        nc.gpsimd.dma_gather(
            wu[:], feat_gap, wu_idx[:], num_idxs=128, num_idxs_reg=128,
            elem_size=4 * C, elem_step=2 * C,
        )

        # weight table columns: [wx0 | wx1 | hy | ly] each NS wide
        WX0, WX1, HY, LY = 0, NS, 2 * NS, 3 * NS
        k0 = 0
        for c, ch in enumerate(CHUNKS):
            # G layout: [128, ch, xtap(2), ytap(2), C]; padded to CHMAX slots
            g = gp.tile([128, CHMAX, 2, 2, C], fdt, tag="g", name=f"g_{c}")
            nc.gpsimd.dma_gather(
                g[:, :ch].rearrange("p k x y c -> p k (x y c)"),
                feat_gap,
                idx_t[:, k0 * 8 : (k0 + ch) * 8],
                num_idxs=ch * 128,
                num_idxs_reg=ch * 128,
                elem_size=4 * C,
                elem_step=2 * C,
            )
            och = op.tile([128, CHMAX * C], fdt, tag="och", name=f"och_{c}")
            for kl in range(ch):
                k = k0 + kl
                # x-combine over both y rows: T[ytap, c] = wx0*G[x0] + wx1*G[x1]
                t = tp.tile([128, 2, C], fdt, tag="tmp", name=f"t_{c}_{kl}")
                nc.scalar.mul(t[:], g[:, kl, 0, :, :], wts_t[:, WX0 + k : WX0 + k + 1])
                nc.vector.scalar_tensor_tensor(
                    t[:], g[:, kl, 1, :, :], wts_t[:, WX1 + k : WX1 + k + 1],
                    t[:], mult, add,
                )
                # y-combine: O = hy*T[0] + ly*T[1]
                u = tp.tile([128, C], fdt, tag="tmpu", name=f"u_{c}_{kl}")
                nc.scalar.mul(u[:], t[:, 1, :], wts_t[:, LY + k : LY + k + 1])
                nc.vector.scalar_tensor_tensor(
                    och[:, kl * C : (kl + 1) * C], t[:, 0, :],
                    wts_t[:, HY + k : HY + k + 1], u[:], mult, add,
                )
            nc.sync.dma_start(
                out.ap()[:, k0 * C : (k0 + ch) * C], och[:, : ch * C]
            )
            k0 += ch

    nc.compile()
    _NC_CACHE = nc
    return nc


def _host_tables(boxes):
    """Numpy f32 replica of the reference's index/weight math.

    Returns None if any box is assigned a level other than 4 (never happens
    with the reference's input distribution), else per-core gather tables.
    """
    f32 = np.float32
    b = boxes.astype(f32)
    box_h = b[..., 2] - b[..., 0]
    box_w = b[..., 3] - b[..., 1]
    area = np.sqrt(box_h * box_w)
    with np.errstate(divide="ignore", invalid="ignore"):
        lev = np.floor(np.log(area / f32(224.0)) / np.log(f32(2.0))) + f32(4.0)
    if not np.all(np.isfinite(lev)):
        return None
    levels = np.clip(lev.astype(np.int32), 4, 64)
    if not np.all(levels == 4):
        return None
    scale = np.exp2(levels.astype(f32))
    bs = b / scale[..., None]
    bh = (box_h / scale).astype(f32)
    bw = (box_w / scale).astype(f32)
    by = (bs[..., 0] - f32(0.5)).astype(f32)
    bx = (bs[..., 1] - f32(0.5)).astype(f32)
    offs = ((np.arange(OUT, dtype=f32) + f32(0.5)) / f32(OUT)).astype(f32)
    gy = (by[..., None] + offs * bh[..., None]).astype(f32)  # [B,N,7]
    gx = (bx[..., None] + offs * bw[..., None]).astype(f32)
    y0 = np.maximum(f32(0.0), np.floor(gy))
    x0 = np.maximum(f32(0.0), np.floor(gx))
    bnd = f32(H - 1)
    y_lo = np.minimum(y0, bnd).astype(np.int32)
    y_hi = np.minimum(y0 + f32(1.0), bnd).astype(np.int32)
    x_lo = np.minimum(x0, bnd).astype(np.int32)
    x_hi = np.minimum(x0 + f32(1.0), bnd).astype(np.int32)
    ly = (gy - y0).astype(f32)
    lx = (gx - x0).astype(f32)
    hy = (f32(1.0) - ly).astype(f32)
    hx = (f32(1.0) - lx).astype(f32)
    # 2-pixel gather base in x; remap x-tap weights onto (xb, xb+1)
    xb = np.minimum(x_lo, W - 2)
    wx0 = hx * (x_lo == xb) + lx * (x_hi == xb)
    wx1 = hx * (x_lo == xb + 1) + lx * (x_hi == xb + 1)
    return y_lo, y_hi, xb, hy, ly, wx0.astype(f32), wx1.astype(f32)


def _feat_pairs(feat0_b):
    """[H*W, 2*C] row-pair layout: row (y*W+x) = [feat[y,x,:], feat[y+1,x,:]]
    (last row duplicates y=127, matching the reference's boundary clamp)."""
    fp = np.empty((H, W, 2, C), dtype=FDT)
    fp[:, :, 0] = feat0_b
    fp[:-1, :, 1] = feat0_b[1:]
    fp[-1, :, 1] = feat0_b[-1]
    return np.ascontiguousarray(fp.reshape(H * W, 2 * C))


def _percore_inputs(featp_by_batch, tables, core):
    y_lo, y_hi, xb, hy, ly, wx0, wx1 = tables
    bat, half = divmod(core, 2)
    sl = slice(half * BOX_PER_CORE, (half + 1) * BOX_PER_CORE)
    ylo = y_lo[bat, sl]  # [128, 7]
    xbs = xb[bat, sl]
    # flat pixel index of the 2x2 block base, [128 box, 7 i, 7 j]
    i0 = (ylo[:, :, None] * W + xbs[:, None, :]).astype(np.int32)

    # gather sequence i = k*128 + box  (k = i_s*7 + j_s)
    seq = np.transpose(i0, (1, 2, 0)).reshape(NIDX).astype(np.int16)
    wr = seq.reshape(WCOLS, 16).T  # [16, WCOLS]
    idx = np.tile(wr, (8, 1))      # replicate across the 8 gpsimd cores

    hys = hy[bat, sl]    # [128, 7] per sample-row i
    lys = ly[bat, sl]
    wx0s = wx0[bat, sl]  # [128, 7] per sample-col j
    wx1s = wx1[bat, sl]
    ones_i = np.ones((1, OUT, 1), dtype=np.float32)
    # expand to [128, NS] with k = i*7 + j
    wx0_k = (ones_i * wx0s[:, None, :]).reshape(128, NS)
    wx1_k = (ones_i * wx1s[:, None, :]).reshape(128, NS)
    ones_j = np.ones((1, 1, OUT), dtype=np.float32)
    hy_k = (hys[:, :, None] * ones_j).reshape(128, NS)
    ly_k = (lys[:, :, None] * ones_j).reshape(128, NS)
    wts = np.concatenate([wx0_k, wx1_k, hy_k, ly_k], axis=1).astype(np.float32)

    return {
        "feat": featp_by_batch[bat],
        "idx": np.ascontiguousarray(idx),
        "wts": np.ascontiguousarray(wts),
    }


def _reference_numpy(feats, boxes):
    """Generic fallback: straight numpy port of the reference (never used
    with the reference input distribution; kept for safety)."""
    f32 = np.float32
    L = len(feats)
    padded = np.zeros((B, L, H, W, C), dtype=f32)
    for i, f in enumerate(feats):
        padded[:, i, : f.shape[1], : f.shape[2], :] = f
    b = boxes.astype(f32)
    box_h = b[..., 2] - b[..., 0]
    box_w = b[..., 3] - b[..., 1]
    area = np.sqrt(box_h * box_w)
    lev = np.floor(np.log(area / f32(224.0)) / np.log(f32(2.0))) + f32(4.0)
    levels = np.clip(lev.astype(np.int32), 4, 64)
    scale = np.exp2(levels.astype(f32))
    bs = b / scale[..., None]
    bh = box_h / scale
    bw = box_w / scale
    yxhw = np.concatenate([bs[..., 0:2], bh[..., None], bw[..., None]], axis=-1)
    lvl = levels - 4
    strides = np.exp2(lvl.astype(f32))
    bnd_h = H / strides - f32(1.0)
    bnd_w = W / strides - f32(1.0)
    by = bnd_w[..., None]  # faithful swap from the reference
    bx = bnd_h[..., None]
    box_y = yxhw[..., 0] - f32(0.5)
    box_x = yxhw[..., 1] - f32(0.5)
    offs = (np.arange(OUT, dtype=f32) + f32(0.5)) / f32(OUT)
    gy = box_y[..., None] + offs * yxhw[..., 2:3]
    gx = box_x[..., None] + offs * yxhw[..., 3:4]
    y0 = np.maximum(f32(0.0), np.floor(gy))
    x0 = np.maximum(f32(0.0), np.floor(gx))
    y01 = np.stack([np.minimum(y0, by), np.minimum(y0 + 1, by)], axis=3).reshape(
        B, N, 2 * OUT
    )
    x01 = np.stack([np.minimum(x0, bx), np.minimum(x0 + 1, bx)], axis=3).reshape(
        B, N, 2 * OUT
    )
    yi = y01.astype(np.int32)
    xi = x01.astype(np.int32)
    bi = np.arange(B)[:, None, None, None]
    li = np.clip(lvl, 0, L - 1)[:, :, None, None]
    gathered = padded[bi, li, yi[:, :, :, None], xi[:, :, None, :]]
    ly = gy - y0
    lx = gx - x0
    hy = 1.0 - ly
    hx = 1.0 - lx
    ky = np.stack([hy, ly], axis=3).reshape(B, N, 2 * OUT, 1)
    kx = np.stack([hx, lx], axis=3).reshape(B, N, 1, 2 * OUT)
    kern = (ky * kx * 4.0).astype(f32)
    weighted = gathered * kern[..., None]
    out = weighted.reshape(B, N, OUT, 2, OUT, 2, C).mean(axis=(3, 5))
    return out.astype(f32)


_TRACE_TMPDIR = None


def _run(in_maps, trace=False):
    from concourse.bass_utils import run_bass_kernel_spmd

    nc = _build_nc()
    kw = {}
    if trace and _TRACE_TMPDIR:
        kw["tmpdir"] = _TRACE_TMPDIR
    return run_bass_kernel_spmd(nc, in_maps, list(range(NCORES)), trace=trace, **kw)


def _kernel_impl(inputs, trace=False):
    feats = [np.asarray(inputs[f"feat{i}"], dtype=np.float32) for i in range(5)]
    boxes = np.asarray(inputs["boxes"], dtype=np.float32)
    tables = _host_tables(boxes)
    if tables is None:
        return _reference_numpy(feats, boxes), None
    featp = [_feat_pairs(feats[0][b]) for b in range(B)]
    in_maps = [_percore_inputs(featp, tables, c) for c in range(NCORES)]
    res = _run(in_maps, trace=trace)
    full = np.empty((B, N, OUT, OUT, C), dtype=np.float32)
    for core in range(NCORES):
        bat, half = divmod(core, 2)
        o = res.results[core]["out"].astype(np.float32).reshape(
            BOX_PER_CORE, OUT, OUT, C
        )
        full[bat, half * BOX_PER_CORE : (half + 1) * BOX_PER_CORE] = o
    return full, res


def kernel(**inputs):
    out, _ = _kernel_impl(inputs)
    return out


def kernel_profiled(**inputs):
    """Like kernel() but with trace=True; returns (output, BassKernelResults)."""
    return _kernel_impl(inputs, trace=True)
